# revision 1
# baseline (speedup 1.0000x reference)
"""PairEmbedding Bass kernel for 8 TRN2 NeuronCores.

out[b,i,j,:] = Co[b,j,:] + Cp[b,i,:] + sep(b,i,j) * w_sep
  Co[j] = se_j @ W1 + [0 | pe_j]
  Cp[i] = se_i @ W2 + b_proj + [pe_i | 0]
  sep(i,j) = ln(|aa_i - aa_j| + 1)
where se = emb_table[seq], pe = pos_table[aa_idx], W1 = W_proj[0:144],
W2 = W_proj[144:288], w_sep = W_proj[288].

Sharding: core c -> batch b = c//2, row block i in [128*(c%2), 128*(c%2)+128),
all 256 j. Per-core output (128, 256, 288) f32.
"""

import math
from contextlib import ExitStack

import numpy as np

from concourse import bacc, bass, mybir, tile
from concourse.bass_utils import run_bass_kernel_spmd

dt = mybir.dt
AF = mybir.ActivationFunctionType
ALU = mybir.AluOpType

B = 4
L = 256
D_PAIR = 288
D_HALF = 144
MAX_LEN = 260
VOCAB = 21
IH = 128          # i rows per core
JG = 8            # j's per output DMA group
N_CORES = 8


def _pos_enc_table() -> np.ndarray:
    idx = np.arange(0, D_HALF, 2, dtype=np.float32)
    t = (np.float32(math.log(10000.0)) * idx) / np.float32(D_HALF)
    denom = np.exp(t, dtype=np.float32)
    pos = np.arange(MAX_LEN, dtype=np.float32)[:, None]
    pe = np.zeros((MAX_LEN, D_HALF), dtype=np.float32)
    pe[:, 0::2] = np.sin(pos / denom, dtype=np.float32)
    pe[:, 1::2] = np.cos(pos / denom, dtype=np.float32)
    return pe


def _bcast(ap_src, nparts: int):
    return bass.AP(
        tensor=ap_src.tensor, offset=ap_src.offset, ap=[[0, nparts], *ap_src.ap]
    )


def build(stage: str = "full", repeat: int = 1, variant: str = "") -> bass.Bass:
    nc = bacc.Bacc("TRN2", target_bir_lowering=False)

    seqb_d = nc.dram_tensor("seqb", [L], dt.int32, kind="ExternalInput")
    seqi_d = nc.dram_tensor("seqi", [IH], dt.int32, kind="ExternalInput")
    aab_d = nc.dram_tensor("aab", [L], dt.int32, kind="ExternalInput")
    aai_d = nc.dram_tensor("aai", [IH], dt.int32, kind="ExternalInput")
    emb_d = nc.dram_tensor("emb", [VOCAB, D_HALF], dt.float32, kind="ExternalInput")
    wp_d = nc.dram_tensor("wp", [D_PAIR + 1, D_PAIR], dt.float32, kind="ExternalInput")
    bp_d = nc.dram_tensor("bp", [D_PAIR], dt.float32, kind="ExternalInput")
    out_d = nc.dram_tensor("out", [IH, L, D_PAIR], dt.float32, kind="ExternalOutput")

    # pos-table gather sources, pre-arranged on host: chunk c of <=128 pos
    # rows on partitions, channel slice [0:144] (posL, pe_i) or [144:288]
    # (posR, pe_j), zero elsewhere.
    pos_np = _pos_enc_table()
    posL_np = np.zeros((128, 3 * D_PAIR), dtype=np.float32)
    posR_np = np.zeros((128, 3 * D_PAIR), dtype=np.float32)
    for c in range(3):
        rows = 128 if c < 2 else MAX_LEN - 256
        chunk = pos_np[c * 128 : c * 128 + rows, :]
        posL_np[0:rows, c * D_PAIR : c * D_PAIR + D_HALF] = chunk
        posR_np[0:rows, c * D_PAIR + D_HALF : (c + 1) * D_PAIR] = chunk
    posL_d = nc.inline_tensor(posL_np, "posL_c")
    posR_d = nc.inline_tensor(posR_np, "posR_c")
    iota_np = (
        np.arange(128, dtype=np.float32)[:, None]
        + 128.0 * np.arange(3, dtype=np.float32)[None, :]
    ).astype(np.float32)
    iota_d = nc.inline_tensor(iota_np, "iota")

    with tile.TileContext(nc) as tc, ExitStack() as ctx:
        persist = ctx.enter_context(tc.tile_pool(name="persist", bufs=1))

        # persistent tiles consumed by the j-loop
        flat_t = persist.tile([2, L * D_PAIR], dt.bfloat16, tag="flat")
        ones_t = persist.tile([2, IH], dt.bfloat16, tag="ones")
        cp_t = persist.tile([IH, D_PAIR], dt.float32, tag="cpt")
        wsep_t = persist.tile([IH, D_PAIR], dt.float32, tag="wsep")
        sep_t = persist.tile([IH, L], dt.float32, tag="sept")

        nc.vector.memset(ones_t, 1.0)

        with ExitStack() as pre:
            scr = pre.enter_context(tc.tile_pool(name="scr", bufs=1))
            psc = pre.enter_context(tc.tile_pool(name="psc", bufs=1, space="PSUM"))

            # ---- input loads ----
            iota_t = scr.tile([128, 3], dt.float32, tag="iota")
            nc.sync.dma_start(iota_t, iota_d[:, :])

            emb_t = scr.tile([VOCAB, D_HALF], dt.float32, tag="emb")
            nc.sync.dma_start(emb_t, emb_d[:, :])

            w1a = scr.tile([128, D_PAIR], dt.float32, tag="w1a")
            nc.sync.dma_start(w1a, wp_d[0:128, :])
            w1b = scr.tile([16, D_PAIR], dt.float32, tag="w1b")
            nc.sync.dma_start(w1b, wp_d[128:144, :])
            w2a = scr.tile([128, D_PAIR], dt.float32, tag="w2a")
            nc.sync.dma_start(w2a, wp_d[144:272, :])
            w2b = scr.tile([16, D_PAIR], dt.float32, tag="w2b")
            nc.sync.dma_start(w2b, wp_d[272:288, :])
            nc.sync.dma_start(wsep_t, _bcast(wp_d[288:289, :], 128))

            bp_t = scr.tile([1, D_PAIR], dt.float32, tag="bp")
            nc.sync.dma_start(bp_t, bp_d[:])

            aaB_i = scr.tile([128, L], dt.int32, tag="aaBi")
            nc.sync.dma_start(aaB_i, _bcast(aab_d[:], 128))
            seqB_i = scr.tile([VOCAB, L], dt.int32, tag="seqBi")
            nc.sync.dma_start(seqB_i, _bcast(seqb_d[:], VOCAB))
            seqI_i = scr.tile([VOCAB, IH], dt.int32, tag="seqIi")
            nc.sync.dma_start(seqI_i, _bcast(seqi_d[:], VOCAB))
            aaIB_i = scr.tile([128, IH], dt.int32, tag="aaIBi")
            nc.sync.dma_start(aaIB_i, _bcast(aai_d[:], 128))
            aaCol_i = scr.tile([IH, 1], dt.int32, tag="aaColi")
            nc.sync.dma_start(aaCol_i, aai_d[:])

            posL = scr.tile([128, 3 * D_PAIR], dt.float32, tag="posL")
            nc.sync.dma_start(posL, posL_d[:, :])
            posR = scr.tile([128, 3 * D_PAIR], dt.float32, tag="posR")
            nc.sync.dma_start(posR, posR_d[:, :])

            # ---- int -> f32 casts ----
            aaB_f = scr.tile([128, L], dt.float32, tag="aaBf")
            nc.vector.tensor_copy(aaB_f, aaB_i)
            seqB_f = scr.tile([VOCAB, L], dt.float32, tag="seqBf")
            nc.vector.tensor_copy(seqB_f, seqB_i)
            seqI_f = scr.tile([VOCAB, IH], dt.float32, tag="seqIf")
            nc.vector.tensor_copy(seqI_f, seqI_i)
            aaIB_f = scr.tile([128, IH], dt.float32, tag="aaIBf")
            nc.vector.tensor_copy(aaIB_f, aaIB_i)
            aaCol_f = scr.tile([IH, 1], dt.float32, tag="aaColf")
            nc.vector.tensor_copy(aaCol_f, aaCol_i)

            # ---- one-hots ----
            ohSeq = scr.tile([VOCAB, L], dt.float32, tag="ohSeq")
            nc.vector.tensor_scalar(
                ohSeq, seqB_f, iota_t[0:VOCAB, 0:1], None, ALU.is_equal
            )
            ohSeqI = scr.tile([VOCAB, IH], dt.float32, tag="ohSeqI")
            nc.vector.tensor_scalar(
                ohSeqI, seqI_f, iota_t[0:VOCAB, 0:1], None, ALU.is_equal
            )
            ohP = []
            ohPi = []
            for c in range(3):
                t = scr.tile([128, L], dt.float32, tag=f"ohP{c}", name=f"ohP{c}")
                nc.vector.tensor_scalar(t, aaB_f, iota_t[:, c : c + 1], None, ALU.is_equal)
                ohP.append(t)
                ti = scr.tile([128, IH], dt.float32, tag=f"ohPi{c}", name=f"ohPi{c}")
                nc.vector.tensor_scalar(
                    ti, aaIB_f, iota_t[:, c : c + 1], None, ALU.is_equal
                )
                ohPi.append(ti)

            # ---- seT = emb^T gathered by seq: (144, L) split 128+16 rows ----
            seT_a_ps = psc.tile([128, L], dt.float32, tag="seTaP")
            nc.tensor.matmul(seT_a_ps, emb_t[:, 0:128], ohSeq, start=True, stop=True)
            seT_b_ps = psc.tile([16, L], dt.float32, tag="seTbP")
            nc.tensor.matmul(
                seT_b_ps, emb_t[:, 128:D_HALF], ohSeq, start=True, stop=True
            )
            seT_a = scr.tile([128, L], dt.float32, tag="seTa")
            nc.vector.tensor_copy(seT_a, seT_a_ps)
            seT_b = scr.tile([16, L], dt.float32, tag="seTb")
            nc.vector.tensor_copy(seT_b, seT_b_ps)

            seTi_a_ps = psc.tile([128, IH], dt.float32, tag="seTiaP")
            nc.tensor.matmul(
                seTi_a_ps, emb_t[:, 0:128], ohSeqI, start=True, stop=True
            )
            seTi_b_ps = psc.tile([16, IH], dt.float32, tag="seTibP")
            nc.tensor.matmul(
                seTi_b_ps, emb_t[:, 128:D_HALF], ohSeqI, start=True, stop=True
            )
            seTi_a = scr.tile([128, IH], dt.float32, tag="seTia")
            nc.vector.tensor_copy(seTi_a, seTi_a_ps)
            seTi_b = scr.tile([16, IH], dt.float32, tag="seTib")
            nc.vector.tensor_copy(seTi_b, seTi_b_ps)

            # ---- Co halves -> bf16 hi/lo -> flat layout on partitions 0/32 ----
            for h in range(2):
                co_ps = psc.tile(
                    [128, D_PAIR], dt.float32, tag=f"co{h}", name=f"co{h}"
                )
                sl = slice(h * 128, (h + 1) * 128)
                nc.tensor.matmul(co_ps, seT_a[:, sl], w1a, start=True, stop=False)
                nc.tensor.matmul(co_ps, seT_b[:, sl], w1b, start=False, stop=False)
                for c in range(3):
                    nc.tensor.matmul(
                        co_ps,
                        ohP[c][:, sl],
                        posR[:, c * D_PAIR : (c + 1) * D_PAIR],
                        start=False,
                        stop=(c == 2),
                    )
                co_hi = scr.tile(
                    [128, D_PAIR], dt.bfloat16, tag=f"cohi{h}", name=f"cohi{h}"
                )
                nc.vector.tensor_copy(co_hi, co_ps)
                co_lo = scr.tile(
                    [128, D_PAIR], dt.bfloat16, tag=f"colo{h}", name=f"colo{h}"
                )
                nc.vector.tensor_sub(co_lo, co_ps, co_hi)
                dst = slice(h * 128 * D_PAIR, (h * 128 + 128) * D_PAIR)
                nc.sync.dma_start(flat_t[0:1, dst], co_hi)
                nc.sync.dma_start(flat_t[1:2, dst], co_lo)

            # ---- Cp = se_i @ W2 + b_proj + [pe_i | 0] ----
            ones_f = scr.tile([1, IH], dt.float32, tag="onesf")
            nc.vector.memset(ones_f, 1.0)
            cp_ps = psc.tile([128, D_PAIR], dt.float32, tag="cpP")
            nc.tensor.matmul(cp_ps, seTi_a, w2a, start=True, stop=False)
            nc.tensor.matmul(cp_ps, seTi_b, w2b, start=False, stop=False)
            for c in range(3):
                nc.tensor.matmul(
                    cp_ps,
                    ohPi[c],
                    posL[:, c * D_PAIR : (c + 1) * D_PAIR],
                    start=False,
                    stop=False,
                )
            nc.tensor.matmul(cp_ps, ones_f, bp_t, start=False, stop=True)
            nc.vector.tensor_copy(cp_t, cp_ps)

            # ---- sep = ln(|aa_j - aa_i| + 1) ----
            dist_t = scr.tile([IH, L], dt.float32, tag="dist")
            nc.vector.tensor_scalar(dist_t, aaB_f, aaCol_f, None, ALU.subtract)
            abs_t = scr.tile([IH, L], dt.float32, tag="abs")
            nc.scalar.activation(abs_t, dist_t, AF.Abs)
            nc.scalar.activation(sep_t, abs_t, AF.Ln, bias=1.0)

        if stage == "setup":
            # dump a few persistent tiles into out rows and stop
            dbg = ctx.enter_context(tc.tile_pool(name="dbg", bufs=1))
            dbf = dbg.tile([IH, D_PAIR], dt.float32, tag="dbf")
            nc.vector.tensor_copy(dbf, cp_t)
            nc.sync.dma_start(out_d[:, 0:1, :], dbf)
            nc.vector.tensor_copy(dbf, wsep_t)
            nc.sync.dma_start(out_d[:, 1:2, :], dbf)
            return nc

        # ---- j loop ----
        psj = ctx.enter_context(tc.tile_pool(name="psj", bufs=8, space="PSUM"))
        obp = ctx.enter_context(tc.tile_pool(name="obp", bufs=2))
        ngroups = int(stage[5:]) if stage.startswith("jloop") else L // JG
        if variant == "dmaonly":
            obs = []
            for k in range(2):
                t = obp.tile([IH, JG * D_PAIR], dt.float32, tag="ob", name="ob")
                nc.vector.memset(t, 0.5)
                obs.append(t)
            for g in range(ngroups * repeat):
                g = g % ngroups
                eng = nc.sync if g % 2 == 0 else nc.scalar
                eng.dma_start(out_d[:, g * JG : (g + 1) * JG, :], obs[g % 2])
            return nc
        for g in range(ngroups * repeat):
            g = g % ngroups
            ob = obp.tile([IH, JG * D_PAIR], dt.float32, tag="ob", name="ob")
            for jj in range(JG):
                j = g * JG + jj
                ps = psj.tile([IH, D_PAIR], dt.float32, tag="ps", name="ps")
                nc.tensor.matmul(
                    ps,
                    ones_t[0:2, :],
                    flat_t[0:2, j * D_PAIR : (j + 1) * D_PAIR],
                    start=True,
                    stop=True,
                )
                osl = ob[:, jj * D_PAIR : (jj + 1) * D_PAIR]
                if variant == "nostt":
                    nc.vector.tensor_copy(osl, ps)
                elif variant == "sttsb":
                    nc.vector.scalar_tensor_tensor(
                        osl, wsep_t, sep_t[:, j : j + 1], cp_t, ALU.mult, ALU.add
                    )
                else:
                    nc.vector.scalar_tensor_tensor(
                        osl, wsep_t, sep_t[:, j : j + 1], ps, ALU.mult, ALU.add
                    )
                if variant not in ("nopool", "nostt", "sttsb"):
                    nc.gpsimd.tensor_add(osl, osl, cp_t)
            if variant != "nodma":
                eng = nc.sync if g % 2 == 0 else nc.scalar
                eng.dma_start(out_d[:, g * JG : (g + 1) * JG, :], ob)

    return nc


_NC_CACHE = []


def make_in_maps(seq, aa_idx, emb_table, W_proj, b_proj):
    seq = np.asarray(seq, dtype=np.int32)
    aa_idx = np.asarray(aa_idx, dtype=np.int32)
    emb_table = np.ascontiguousarray(np.asarray(emb_table, dtype=np.float32))
    W_proj = np.ascontiguousarray(np.asarray(W_proj, dtype=np.float32))
    b_proj = np.ascontiguousarray(np.asarray(b_proj, dtype=np.float32))
    in_maps = []
    for c in range(N_CORES):
        b, ih = c // 2, c % 2
        in_maps.append(
            {
                "seqb": np.ascontiguousarray(seq[b]),
                "seqi": np.ascontiguousarray(seq[b, ih * IH : (ih + 1) * IH]),
                "aab": np.ascontiguousarray(aa_idx[b]),
                "aai": np.ascontiguousarray(aa_idx[b, ih * IH : (ih + 1) * IH]),
                "emb": emb_table,
                "wp": W_proj,
                "bp": b_proj,
            }
        )
    return in_maps


def gather_out(results) -> np.ndarray:
    out = np.empty((B, L, L, D_PAIR), dtype=np.float32)
    for c in range(N_CORES):
        b, ih = c // 2, c % 2
        out[b, ih * IH : (ih + 1) * IH] = np.asarray(results[c]["out"])
    return out


def kernel(seq, aa_idx, emb_table, W_proj, b_proj) -> np.ndarray:
    if not _NC_CACHE:
        nc = build()
        nc.finalize()
        _NC_CACHE.append(nc)
    nc = _NC_CACHE[0]
    in_maps = make_in_maps(seq, aa_idx, emb_table, W_proj, b_proj)
    res = run_bass_kernel_spmd(nc, in_maps, core_ids=list(range(N_CORES)))
    return gather_out(res.results)



# revision 3
# speedup vs baseline: 2.7020x; 2.7020x over previous
"""PairEmbedding Bass kernel for 8 TRN2 NeuronCores.

out[b,i,j,:] = Cp[b,i,:] + Co[b,j,:] + sep(b,i,j) * w_sep
  Co[j] = se_j @ W1 + [0 | pe_j]
  Cp[i] = se_i @ W2 + b_proj + [pe_i | 0]
  sep(i,j) = ln(|aa_i - aa_j| + 1)
where se = emb_table[seq], pe = pos_table[aa_idx], W1 = W_proj[0:144],
W2 = W_proj[144:288], w_sep = W_proj[288].

Sharding: core c -> batch b = c//2, row block i in [128*(c%2), 128*(c%2)+128),
all 256 j. Per-core output (128, 256, 288) bf16 on device, upcast to f32 on
host after gather.

Engine plan (per core):
- PE seeds PSUM bank b (j = b, b = 0..7) with Cp (identity matmuls of bf16
  hi/lo) + X_b, then one K=6 delta matmul per j >= 8 accumulates
  X_j - X_{j-8} on bank j%8, where X_j = Co_hi[j] + Co_lo[j] + sep_j*wsep.
  Negation comes from -1 rows in the stationary operand; the subtracted
  values are bit-identical bf16 products, so cancellation is exact.
- ACT converts banks 0-3 (f32 PSUM -> bf16 SBUF), DVE converts banks 4-7,
  in parallel (different banks). One instr per 4 j's, strided PSUM read.
- SP issues one output DMA per 8 j's (128 x 4608B bf16).
- j-indexed matmul operands (Co rows flattened j-major, sepT rows) are
  built in 4 phases of 64 j's, double-buffered, via gpsimd-issued
  SBUF->SBUF flatten DMAs that overlap compute.
"""

import math
from contextlib import ExitStack

import numpy as np

from concourse import bacc, bass, mybir, tile
from concourse.bass_utils import run_bass_kernel_spmd

dt = mybir.dt
AF = mybir.ActivationFunctionType
ALU = mybir.AluOpType

B = 4
L = 256
D_PAIR = 288
D_HALF = 144
MAX_LEN = 260
VOCAB = 21
IH = 128          # i rows per core
N_CORES = 8
PJ = 64           # j's per phase
NPH = L // PJ     # 4 phases
SHIFT = 8         # delta distance == number of PSUM banks
BANKW = 512       # f32 slots per PSUM bank


def _pos_enc_table() -> np.ndarray:
    idx = np.arange(0, D_HALF, 2, dtype=np.float32)
    t = (np.float32(math.log(10000.0)) * idx) / np.float32(D_HALF)
    denom = np.exp(t, dtype=np.float32)
    pos = np.arange(MAX_LEN, dtype=np.float32)[:, None]
    pe = np.zeros((MAX_LEN, D_HALF), dtype=np.float32)
    pe[:, 0::2] = np.sin(pos / denom, dtype=np.float32)
    pe[:, 1::2] = np.cos(pos / denom, dtype=np.float32)
    return pe


def _bcast(ap_src, nparts: int):
    return bass.AP(
        tensor=ap_src.tensor, offset=ap_src.offset, ap=[[0, nparts], *ap_src.ap]
    )


def _rep(ap_src, nrep: int):
    # repeat a [1, N] SBUF row nrep times along free dim (stride-0 outer)
    return bass.AP(
        tensor=ap_src.tensor,
        offset=ap_src.offset,
        ap=[ap_src.ap[0], [0, nrep], ap_src.ap[1]],
    )


def _banks(ps_ap, bank0: int, nbank: int, ncol: int):
    # strided read of `ncol` leading f32 of `nbank` consecutive PSUM banks
    return bass.AP(
        tensor=ps_ap.tensor,
        offset=ps_ap.offset + bank0 * BANKW,
        ap=[ps_ap.ap[0], [BANKW, nbank], [1, ncol]],
    )


def build(repeat: int = 1, variant: str = "") -> bass.Bass:
    nc = bacc.Bacc("TRN2", target_bir_lowering=False)

    seqb_d = nc.dram_tensor("seqb", [L], dt.int32, kind="ExternalInput")
    seqi_d = nc.dram_tensor("seqi", [IH], dt.int32, kind="ExternalInput")
    aab_d = nc.dram_tensor("aab", [L], dt.int32, kind="ExternalInput")
    aai_d = nc.dram_tensor("aai", [IH], dt.int32, kind="ExternalInput")
    emb_d = nc.dram_tensor("emb", [VOCAB, D_HALF], dt.float32, kind="ExternalInput")
    wp_d = nc.dram_tensor("wp", [D_PAIR + 1, D_PAIR], dt.float32, kind="ExternalInput")
    bp_d = nc.dram_tensor("bp", [D_PAIR], dt.float32, kind="ExternalInput")
    out_d = nc.dram_tensor("out", [IH, L, D_PAIR], dt.bfloat16, kind="ExternalOutput")

    # pos-table gather sources, pre-arranged on host: chunk c of <=128 pos
    # rows on partitions, channel slice [0:144] (posL, pe_i) or [144:288]
    # (posR, pe_j), zero elsewhere.
    pos_np = _pos_enc_table()
    posL_np = np.zeros((128, 3 * D_PAIR), dtype=np.float32)
    posR_np = np.zeros((128, 3 * D_PAIR), dtype=np.float32)
    for c in range(3):
        rows = 128 if c < 2 else MAX_LEN - 256
        chunk = pos_np[c * 128 : c * 128 + rows, :]
        posL_np[0:rows, c * D_PAIR : c * D_PAIR + D_HALF] = chunk
        posR_np[0:rows, c * D_PAIR + D_HALF : (c + 1) * D_PAIR] = chunk
    posL_d = nc.inline_tensor(posL_np, "posL_c")
    posR_d = nc.inline_tensor(posR_np, "posR_c")
    iota_np = (
        np.arange(128, dtype=np.float32)[:, None]
        + 128.0 * np.arange(3, dtype=np.float32)[None, :]
    ).astype(np.float32)
    iota_d = nc.inline_tensor(iota_np, "iota")
    eye_d = nc.inline_tensor(np.eye(128, dtype=np.float32), "eye")

    with tile.TileContext(nc) as tc, ExitStack() as ctx:
        persist = ctx.enter_context(tc.tile_pool(name="persist", bufs=1))

        # persistent tiles consumed by the j-loop
        rhs_b = [
            persist.tile([6, PJ * D_PAIR], dt.bfloat16, tag=f"rhs{k}", name=f"rhs{k}") for k in range(2)
        ]
        lhs_b = [
            persist.tile([6, PJ * IH], dt.bfloat16, tag=f"lhs{k}", name=f"lhs{k}") for k in range(2)
        ]
        co_hi = [
            persist.tile([128, D_PAIR], dt.bfloat16, tag=f"cohi{c}", name=f"cohi{c}") for c in range(2)
        ]
        co_lo = [
            persist.tile([128, D_PAIR], dt.bfloat16, tag=f"colo{c}", name=f"colo{c}") for c in range(2)
        ]
        sepT = [
            persist.tile([128, IH], dt.bfloat16, tag=f"sepT{c}", name=f"sepT{c}") for c in range(2)
        ]
        cp_hi = persist.tile([128, D_PAIR], dt.bfloat16, tag="cphi")
        cp_lo = persist.tile([128, D_PAIR], dt.bfloat16, tag="cplo")
        id_bf = persist.tile([128, 128], dt.bfloat16, tag="idbf")
        ones_sq = persist.tile([128, 128], dt.bfloat16, tag="onessq")
        neg_sq = persist.tile([128, 128], dt.bfloat16, tag="negsq")
        wsep_bf = persist.tile([1, D_PAIR], dt.bfloat16, tag="wsepbf")
        nwsep_bf = persist.tile([1, D_PAIR], dt.bfloat16, tag="nwsepbf")

        nc.vector.memset(ones_sq, 1.0)
        nc.vector.memset(neg_sq, -1.0)

        with ExitStack() as pre:
            scr = pre.enter_context(tc.tile_pool(name="scr", bufs=1))
            psc = pre.enter_context(tc.tile_pool(name="psc", bufs=1, space="PSUM"))

            # ---- input loads ----
            iota_t = scr.tile([128, 3], dt.float32, tag="iota")
            nc.sync.dma_start(iota_t, iota_d[:, :])
            eye_t = scr.tile([128, 128], dt.float32, tag="eye")
            nc.sync.dma_start(eye_t, eye_d[:, :])

            emb_t = scr.tile([VOCAB, D_HALF], dt.float32, tag="emb")
            nc.sync.dma_start(emb_t, emb_d[:, :])

            w1a = scr.tile([128, D_PAIR], dt.float32, tag="w1a")
            nc.sync.dma_start(w1a, wp_d[0:128, :])
            w1b = scr.tile([16, D_PAIR], dt.float32, tag="w1b")
            nc.sync.dma_start(w1b, wp_d[128:144, :])
            w2a = scr.tile([128, D_PAIR], dt.float32, tag="w2a")
            nc.sync.dma_start(w2a, wp_d[144:272, :])
            w2b = scr.tile([16, D_PAIR], dt.float32, tag="w2b")
            nc.sync.dma_start(w2b, wp_d[272:288, :])
            wsep_f = scr.tile([1, D_PAIR], dt.float32, tag="wsepf")
            nc.sync.dma_start(wsep_f, wp_d[288:289, :])

            bp_t = scr.tile([1, D_PAIR], dt.float32, tag="bp")
            nc.sync.dma_start(bp_t, bp_d[:])

            seqB_i = scr.tile([VOCAB, L], dt.int32, tag="seqBi")
            nc.sync.dma_start(seqB_i, _bcast(seqb_d[:], VOCAB))
            seqI_i = scr.tile([VOCAB, IH], dt.int32, tag="seqIi")
            nc.sync.dma_start(seqI_i, _bcast(seqi_d[:], VOCAB))
            aaB_i = scr.tile([128, L], dt.int32, tag="aaBi")
            nc.sync.dma_start(aaB_i, _bcast(aab_d[:], 128))
            aaIB_i = scr.tile([128, IH], dt.int32, tag="aaIBi")
            nc.sync.dma_start(aaIB_i, _bcast(aai_d[:], 128))
            # aa_j columns for the transposed sep: chunk c -> [128, 1]
            aaCol_i = [
                scr.tile([128, 1], dt.int32, tag=f"aaCol{c}", name=f"aaCol{c}")
                for c in range(2)
            ]
            for c in range(2):
                nc.sync.dma_start(aaCol_i[c], aab_d[c * 128 : (c + 1) * 128])

            posL = scr.tile([128, 3 * D_PAIR], dt.float32, tag="posL")
            nc.sync.dma_start(posL, posL_d[:, :])
            posR = scr.tile([128, 3 * D_PAIR], dt.float32, tag="posR")
            nc.sync.dma_start(posR, posR_d[:, :])

            # ---- int -> f32 casts ----
            seqB_f = scr.tile([VOCAB, L], dt.float32, tag="seqBf")
            nc.vector.tensor_copy(seqB_f, seqB_i)
            seqI_f = scr.tile([VOCAB, IH], dt.float32, tag="seqIf")
            nc.vector.tensor_copy(seqI_f, seqI_i)
            aaB_f = scr.tile([128, L], dt.float32, tag="aaBf")
            nc.vector.tensor_copy(aaB_f, aaB_i)
            aaIB_f = scr.tile([128, IH], dt.float32, tag="aaIBf")
            nc.vector.tensor_copy(aaIB_f, aaIB_i)
            aaCol_f = [
                scr.tile([128, 1], dt.float32, tag=f"aaColf{c}", name=f"aaColf{c}")
                for c in range(2)
            ]
            for c in range(2):
                nc.vector.tensor_copy(aaCol_f[c], aaCol_i[c])

            # ---- one-hots ----
            ohSeq = scr.tile([VOCAB, L], dt.float32, tag="ohSeq")
            nc.vector.tensor_scalar(
                ohSeq, seqB_f, iota_t[0:VOCAB, 0:1], None, ALU.is_equal
            )
            ohSeqI = scr.tile([VOCAB, IH], dt.float32, tag="ohSeqI")
            nc.vector.tensor_scalar(
                ohSeqI, seqI_f, iota_t[0:VOCAB, 0:1], None, ALU.is_equal
            )
            ohP = []
            ohPi = []
            for c in range(3):
                t = scr.tile([128, L], dt.float32, tag=f"ohP{c}", name=f"ohP{c}")
                nc.vector.tensor_scalar(t, aaB_f, iota_t[:, c : c + 1], None, ALU.is_equal)
                ohP.append(t)
                ti = scr.tile([128, IH], dt.float32, tag=f"ohPi{c}", name=f"ohPi{c}")
                nc.vector.tensor_scalar(
                    ti, aaIB_f, iota_t[:, c : c + 1], None, ALU.is_equal
                )
                ohPi.append(ti)

            # ---- seT = emb^T gathered by seq: (144, L) split 128+16 rows ----
            seT_a_ps = psc.tile([128, L], dt.float32, tag="seTaP")
            nc.tensor.matmul(seT_a_ps, emb_t[:, 0:128], ohSeq, start=True, stop=True)
            seT_b_ps = psc.tile([16, L], dt.float32, tag="seTbP")
            nc.tensor.matmul(
                seT_b_ps, emb_t[:, 128:D_HALF], ohSeq, start=True, stop=True
            )
            seT_a = scr.tile([128, L], dt.float32, tag="seTa")
            nc.vector.tensor_copy(seT_a, seT_a_ps)
            seT_b = scr.tile([16, L], dt.float32, tag="seTb")
            nc.vector.tensor_copy(seT_b, seT_b_ps)

            seTi_a_ps = psc.tile([128, IH], dt.float32, tag="seTiaP")
            nc.tensor.matmul(
                seTi_a_ps, emb_t[:, 0:128], ohSeqI, start=True, stop=True
            )
            seTi_b_ps = psc.tile([16, IH], dt.float32, tag="seTibP")
            nc.tensor.matmul(
                seTi_b_ps, emb_t[:, 128:D_HALF], ohSeqI, start=True, stop=True
            )
            seTi_a = scr.tile([128, IH], dt.float32, tag="seTia")
            nc.vector.tensor_copy(seTi_a, seTi_a_ps)
            seTi_b = scr.tile([16, IH], dt.float32, tag="seTib")
            nc.vector.tensor_copy(seTi_b, seTi_b_ps)

            # ---- Co chunks (j on partitions): hi/lo bf16 ----
            for h in range(2):
                co_ps = psc.tile(
                    [128, D_PAIR], dt.float32, tag=f"co{h}", name=f"co{h}"
                )
                sl = slice(h * 128, (h + 1) * 128)
                nc.tensor.matmul(co_ps, seT_a[:, sl], w1a, start=True, stop=False)
                nc.tensor.matmul(co_ps, seT_b[:, sl], w1b, start=False, stop=False)
                for c in range(3):
                    nc.tensor.matmul(
                        co_ps,
                        ohP[c][:, sl],
                        posR[:, c * D_PAIR : (c + 1) * D_PAIR],
                        start=False,
                        stop=(c == 2),
                    )
                nc.vector.tensor_copy(co_hi[h], co_ps)
                nc.vector.tensor_sub(co_lo[h], co_ps, co_hi[h])

            # ---- Cp = se_i @ W2 + b_proj + [pe_i | 0]: hi/lo bf16 ----
            ones_f = scr.tile([1, IH], dt.float32, tag="onesf")
            nc.vector.memset(ones_f, 1.0)
            cp_ps = psc.tile([128, D_PAIR], dt.float32, tag="cpP")
            nc.tensor.matmul(cp_ps, seTi_a, w2a, start=True, stop=False)
            nc.tensor.matmul(cp_ps, seTi_b, w2b, start=False, stop=False)
            for c in range(3):
                nc.tensor.matmul(
                    cp_ps,
                    ohPi[c],
                    posL[:, c * D_PAIR : (c + 1) * D_PAIR],
                    start=False,
                    stop=False,
                )
            nc.tensor.matmul(cp_ps, ones_f, bp_t, start=False, stop=True)
            nc.vector.tensor_copy(cp_hi, cp_ps)
            nc.vector.tensor_sub(cp_lo, cp_ps, cp_hi)

            # ---- sepT[jp, i] = ln(|aa_i - aa_jp| + 1), bf16, 2 chunks ----
            for c in range(2):
                dT = scr.tile([128, IH], dt.float32, tag=f"dT{c}", name=f"dT{c}")
                nc.vector.tensor_scalar(dT, aaIB_f, aaCol_f[c], None, ALU.subtract)
                aT = scr.tile([128, IH], dt.float32, tag=f"aT{c}", name=f"aT{c}")
                nc.scalar.activation(aT, dT, AF.Abs)
                sT = scr.tile([128, IH], dt.float32, tag=f"sT{c}", name=f"sT{c}")
                nc.scalar.activation(sT, aT, AF.Ln, bias=1.0)
                nc.vector.tensor_copy(sepT[c], sT)

            # ---- small bf16 conversions ----
            nc.vector.tensor_copy(id_bf, eye_t)
            nc.vector.tensor_copy(wsep_bf, wsep_f)
            nc.vector.tensor_scalar(nwsep_bf, wsep_f, -1.0, None, ALU.mult)

        # ---- static rows of the double-buffered matmul operands ----
        # rhs rows: 0=co_hi, 1=co_lo, 2=wsep rep, 3=co_hi shifted, 4=co_lo
        # shifted, 5=-wsep rep.  lhs rows: 0,1=+1, 2=sepT_j, 3,4=-1,
        # 5=sepT_{j-8}.
        for k in range(2):
            nc.gpsimd.dma_start(rhs_b[k][2:3, :], _rep(wsep_bf[0:1, :], PJ))
            nc.gpsimd.dma_start(rhs_b[k][5:6, :], _rep(nwsep_bf[0:1, :], PJ))
            nc.gpsimd.dma_start(lhs_b[k][0:1, :], ones_sq[0:PJ, :])
            nc.gpsimd.dma_start(lhs_b[k][1:2, :], ones_sq[0:PJ, :])
            nc.gpsimd.dma_start(lhs_b[k][3:4, :], neg_sq[0:PJ, :])
            nc.gpsimd.dma_start(lhs_b[k][4:5, :], neg_sq[0:PJ, :])

        # helper: write flattened rows [r0, r0+n) of (src chunks) into row
        # `row` of tile `dst`, at j-slot `slot0` (in units of `w` columns).
        def flatten(dst, row, slot0, r0, n, srcs, w):
            # split on the 128-row chunk boundary of the source tiles
            while n > 0:
                c, r = divmod(r0, 128)
                take = min(n, 128 - r)
                nc.gpsimd.dma_start(
                    dst[row : row + 1, slot0 * w : (slot0 + take) * w],
                    srcs[c][r : r + take, :],
                )
                r0 += take
                slot0 += take
                n -= take

        def build_phase(p):
            k = p % 2
            j0 = p * PJ
            flatten(rhs_b[k], 0, 0, j0, PJ, co_hi, D_PAIR)
            flatten(rhs_b[k], 1, 0, j0, PJ, co_lo, D_PAIR)
            flatten(lhs_b[k], 2, 0, j0, PJ, sepT, IH)
            if p == 0:
                flatten(rhs_b[k], 3, SHIFT, 0, PJ - SHIFT, co_hi, D_PAIR)
                flatten(rhs_b[k], 4, SHIFT, 0, PJ - SHIFT, co_lo, D_PAIR)
                flatten(lhs_b[k], 5, SHIFT, 0, PJ - SHIFT, sepT, IH)
            else:
                flatten(rhs_b[k], 3, 0, j0 - SHIFT, PJ, co_hi, D_PAIR)
                flatten(rhs_b[k], 4, 0, j0 - SHIFT, PJ, co_lo, D_PAIR)
                flatten(lhs_b[k], 5, 0, j0 - SHIFT, PJ, sepT, IH)

        # ---- j loop ----
        psj = ctx.enter_context(tc.tile_pool(name="psj", bufs=1, space="PSUM"))
        ps_all = psj.tile([128, 8 * BANKW], dt.float32, tag="psall")
        obp = ctx.enter_context(tc.tile_pool(name="obp", bufs=3))

        if variant == "dmaonly":
            obs = []
            for kk in range(2):
                t = obp.tile([IH, 8 * D_PAIR], dt.bfloat16, tag="ob", name="ob")
                nc.vector.memset(t, 0.5)
                obs.append(t)
            for r in range(repeat):
                for g in range(L // 8):
                    nc.sync.dma_start(out_d[:, g * 8 : (g + 1) * 8, :], obs[g % 2])
            return nc

        for r in range(repeat):
            for p in range(NPH):
                build_phase(p)
                k = p % 2
                for t8 in range(PJ // 8):
                    ob = obp.tile([IH, 8 * D_PAIR], dt.bfloat16, tag="ob", name="ob")
                    for jj in range(8):
                        j = p * PJ + t8 * 8 + jj
                        cc = j - p * PJ
                        bank = j % 8
                        osl = ps_all[:, bank * BANKW : bank * BANKW + D_PAIR]
                        if j < 8:
                            nc.tensor.matmul(
                                osl, id_bf, cp_hi, start=True, stop=False
                            )
                            nc.tensor.matmul(
                                osl, id_bf, cp_lo, start=False, stop=False
                            )
                            nc.tensor.matmul(
                                osl,
                                lhs_b[k][0:3, cc * IH : (cc + 1) * IH],
                                rhs_b[k][0:3, cc * D_PAIR : (cc + 1) * D_PAIR],
                                start=False,
                                stop=True,
                            )
                        else:
                            nc.tensor.matmul(
                                osl,
                                lhs_b[k][0:6, cc * IH : (cc + 1) * IH],
                                rhs_b[k][0:6, cc * D_PAIR : (cc + 1) * D_PAIR],
                                start=False,
                                stop=True,
                                skip_group_check=True,
                            )
                        if variant == "mmonly":
                            continue
                    if variant == "mmonly":
                        continue
                    # moves: ACT on banks 0-3, DVE on banks 4-7
                    nc.scalar.activation(
                        ob[:, 0 : 4 * D_PAIR], _banks(ps_all, 0, 4, D_PAIR), AF.Copy
                    )
                    nc.vector.tensor_copy(
                        ob[:, 4 * D_PAIR : 8 * D_PAIR], _banks(ps_all, 4, 4, D_PAIR)
                    )
                    g = p * (PJ // 8) + t8
                    nc.sync.dma_start(out_d[:, g * 8 : (g + 1) * 8, :], ob)

    return nc


_NC_CACHE = []


def make_in_maps(seq, aa_idx, emb_table, W_proj, b_proj):
    seq = np.asarray(seq, dtype=np.int32)
    aa_idx = np.asarray(aa_idx, dtype=np.int32)
    emb_table = np.ascontiguousarray(np.asarray(emb_table, dtype=np.float32))
    W_proj = np.ascontiguousarray(np.asarray(W_proj, dtype=np.float32))
    b_proj = np.ascontiguousarray(np.asarray(b_proj, dtype=np.float32))
    in_maps = []
    for c in range(N_CORES):
        b, ih = c // 2, c % 2
        in_maps.append(
            {
                "seqb": np.ascontiguousarray(seq[b]),
                "seqi": np.ascontiguousarray(seq[b, ih * IH : (ih + 1) * IH]),
                "aab": np.ascontiguousarray(aa_idx[b]),
                "aai": np.ascontiguousarray(aa_idx[b, ih * IH : (ih + 1) * IH]),
                "emb": emb_table,
                "wp": W_proj,
                "bp": b_proj,
            }
        )
    return in_maps


def gather_out(results) -> np.ndarray:
    out = np.empty((B, L, L, D_PAIR), dtype=np.float32)
    for c in range(N_CORES):
        b, ih = c // 2, c % 2
        out[b, ih * IH : (ih + 1) * IH] = np.asarray(results[c]["out"]).astype(
            np.float32
        )
    return out


def kernel(seq, aa_idx, emb_table, W_proj, b_proj) -> np.ndarray:
    if not _NC_CACHE:
        nc = build()
        nc.finalize()
        _NC_CACHE.append(nc)
    nc = _NC_CACHE[0]
    in_maps = make_in_maps(seq, aa_idx, emb_table, W_proj, b_proj)
    res = run_bass_kernel_spmd(nc, in_maps, core_ids=list(range(N_CORES)))
    return gather_out(res.results)
